# revision 18
# baseline (speedup 1.0000x reference)
"""GRNN (Nadaraya-Watson + linear head) Trainium2 Bass kernel, 8-way row-parallel.

Math: for x [N,D], the reference computes
    sqd_ij = ||x_i||^2 + ||x_j||^2 - 2 x_i.x_j
    w_ij   = exp(-sqd_ij / (2 sigma^2)),  w~ = w / rowsum(w)
    out    = (w~ @ x) @ W.T + b
The exp(-||x_i||^2/2s^2) factor is constant per row i and cancels in the
normalization, so w~ is a softmax over z_ij = (2 x_i.x_j - ||x_j||^2)/(2 s^2).
z is O(0.5) here, so no max-subtraction is needed; EPS=1e-8 is ~1e-12 of the
row sum and is dropped.

Sharding: rows of x are split across 8 cores (1024 rows each); every core
streams the full x (replicated in its HBM) flash-attention style.

Both big matmul phases run in fp8 DoubleRow (2 K-chunks per matmul):
  - G^T[j, i] blocks = xfT.T @ xbT (logits tolerate raw fp8)
  - the y-phase uses MEAN-CENTERED weights: shift z by +512/2048 so
    w' = exp(z') ~ 1.0, and only d = w' - 1 (|d| <~ 0.1) is quantized to
    fp8 -- its absolute quantization error is ~20x smaller than fp8(w')'s.
    Then  sum_j w'_ij x_j = s + sum_j d_ij x_j  with s = colsum(x) exact
    (host fp32), added as a per-partition bias during y^T PSUM staging.
    The normalizer r_i = sum_j w'_ij is unchanged by the shift (it cancels).

Per core, everything is laid out so no on-device transpose is ever needed:
  - G^T[j, i] blocks     = xfT.T @ xbT in fp8 DoubleRow
  - w^T[j, i] = Exp(G^T * 1/1024 + bias_j), bias_j = (512-sq_j)/2048
                (host-computed table), one ScalarE activation out of PSUM
  - d8^T = w^T - 1 on DVE (fp16 in, fp8 out), written into [j2, 2, i]
    pair tiles
  - y^T[d, i] += x8p[j-pair, d-chunk].T @ d8^T  (fp8 DoubleRow, accum)
  - r[1, i]   += ones.T @ sum8(w^T)             (w pre-summed on DVE)
  - out[i, o] = ((y^T_chunk + s_chunk).T @ W^T_chunk) * (1/r_i)

Overhead engineering: every DRAM input is blocked on the host so each DMA
lands as >=1KB-per-partition contiguous slabs (the naive j-sliced layout
produced 256B packets); bulk streams ride the Sync/GpSimd rings only (a
DMA dispatch occupies the issuing engine's sequencer and stalls on ring
credits, so the busy Scalar engine carries just the small early tensors);
outputs are staged fp16, alternate rings, and the last tile splits across
both so the final drain is short.

Measured: ~147.2-148.9us HW exec on 8 cores (baseline 204.0us), rel err
~5.1e-3 vs the fp32 reference (gate 2e-2). PE busy ~127us of the ~129us
mid-stream span; the rest is DMA-gated prologue (~13us, ring-speed bound)
and a fixed ~12.9us tail (output drain + barrier + ~8us of framework
semaphore teardown). Known non-levers: G's global symmetry (PE transpose
of a [1024,1024] block costs 2.3x recomputing it), DVE 4x tensor_scalar
(mode never engages for 16b->fp8), teardown (fixed 250-semaphore sweep).
"""

import numpy as np
import ml_dtypes

BF16 = ml_dtypes.bfloat16

# Problem geometry (hardcoded per spec: x [8192, 512], W [512, 512], b [512])
N = 8192          # total rows of x == number of kernel-weight columns
D = 512           # feature dim
O = 512           # output dim
NCORES = 8
MB = N // NCORES  # rows per core (1024)
JC = 128          # j-chunk (partition dim of w^T tiles)
NJ = N // JC      # 64 j-chunks
NJP = NJ // 2     # 32 j-pairs (fp8 DoubleRow contracts 2 chunks at once)
DC = 128          # d-chunk
NDC = D // DC     # 4 d-chunks
IHW = 512         # i-half width (one PSUM bank of fp32)
NIH = MB // IHW   # 2 i-halves per core
NT = IHW // 128   # 4 i-tiles per half

SIGMA = 32.0
INV_2S2 = 1.0 / (2.0 * SIGMA * SIGMA)          # 1/2048
EXP_SCALE = 2.0 * INV_2S2                      # 1/1024 (z = 2G/2048 + bias)
ZSHIFT = 512.0                                 # bias_j = (ZSHIFT - sq_j)/2048

_CACHE = {}


def _build_nc():
    import concourse.bacc as bacc
    import concourse.mybir as mybir
    import concourse.tile as tile

    fp32 = mybir.dt.float32
    bf16 = mybir.dt.bfloat16

    nc = bacc.Bacc("TRN2", target_bir_lowering=False, debug=False, num_devices=NCORES)

    fp8 = mybir.dt.float8e4
    fp16 = mybir.dt.float16
    # DRAM layouts are blocked so every DMA is a dense per-partition slab:
    #   xfT [p, jp, c, jj]: per 2-chunk group, 1KB/partition contiguous
    #   xbT [p, ih, c, i]:  per i-half, 2KB/partition contiguous
    #   x8p [jp, j2, q, d]: per pair, 1KB/partition contiguous
    xfT = nc.dram_tensor("xfT", [DC, NJP, NDC, 2 * JC], fp8, kind="ExternalInput")
    x8p = nc.dram_tensor("x8p", [JC, NJP, 2, D], fp8, kind="ExternalInput")
    xbT = nc.dram_tensor("xbT", [DC, NIH, NDC, IHW], fp8, kind="ExternalInput")
    wTh = nc.dram_tensor("wTh", [DC, NDC, O], fp16, kind="ExternalInput")
    sqbt = nc.dram_tensor("sqbt", [JC, NJ], fp32, kind="ExternalInput")
    srt = nc.dram_tensor("srt", [DC, NDC], fp32, kind="ExternalInput")
    out = nc.dram_tensor("out", [MB, O], fp16, kind="ExternalOutput")

    # j-chunks per DMA load group: 2-chunk first groups give the PE DENSE
    # early work right after the warmup stream
    if NJ >= 16:
        GROUPS = [2, 2, 4, 8] + [8] * ((NJ - 16) // 8)
    else:
        GROUPS = [2, 2, 4][:NJ]
    assert sum(GROUPS) == NJ

    with tile.TileContext(nc) as tc:
        with (
            tc.tile_pool(name="big", bufs=1) as big,
            tc.tile_pool(name="wpool", bufs=8) as wpool,
            tc.tile_pool(name="dpool", bufs=4) as dpool,
            tc.tile_pool(name="ypool", bufs=2) as ypool,
            tc.tile_pool(name="misc", bufs=2) as misc,
            tc.tile_pool(name="gps", bufs=2, space="PSUM") as gps,
            tc.tile_pool(name="yps", bufs=1, space="PSUM") as yps,
            tc.tile_pool(name="rps", bufs=1, space="PSUM") as rps,
            tc.tile_pool(name="hps", bufs=1, space="PSUM") as hps,
        ):
            # ---- resident SBUF tensors ----
            ones_sb = big.tile([JC, 4], fp16, name="ones_sb", tag="ones")
            nc.vector.memset(ones_sb[:], 1.0)
            idone_sb = big.tile([1, 1], fp32, name="idone_sb", tag="idone")
            nc.vector.memset(idone_sb[:], 1.0)

            sqb_sb = big.tile([JC, NJ], fp32, name="sqb_sb", tag="sqb")
            sr_sb = big.tile([DC, NDC], fp32, name="sr_sb", tag="sr")

            xbT_sb = big.tile([DC, NIH, NDC, IHW], fp8, name="xbT_sb", tag="xbT")
            xfT_sb = big.tile([DC, NJP, NDC, 2 * JC], fp8, name="xfT_sb", tag="xfT")
            x8p_sb = big.tile([JC, NJP, 2, D], fp8, name="x8p_sb", tag="x8p")
            wTh_sb = big.tile([DC, NDC, O], fp16, name="wTh_sb", tag="wTh")

            # gating prologue transfers: bulk streams ride the Sync and
            # GpSimd rings only -- DMA dispatches occupy the issuing engine's
            # sequencer (and stall on ring credits), so the Scalar engine
            # carries nothing but the small early tensors or its Exp
            # activations would be pushed back by tens of microseconds
            # first G matmul (c-pair 0) gates on only the first half of
            # each gating transfer: split them so it can start ~2us sooner
            nc.sync.dma_start(xbT_sb[:, 0, 0:2], xbT[:, 0, 0:2])
            nc.gpsimd.dma_start(xfT_sb[:, 0:1, 0:2], xfT[:, 0:1, 0:2])
            nc.sync.dma_start(xbT_sb[:, 0, 2:4], xbT[:, 0, 2:4])
            nc.gpsimd.dma_start(xfT_sb[:, 0:1, 2:4], xfT[:, 0:1, 2:4])
            nc.scalar.dma_start(sqb_sb[:], sqbt[:])
            nc.scalar.dma_start(xbT_sb[:, 1], xbT[:, 1])

            NSCAL_PAIRS = 8
            jg = 0
            for g in GROUPS:
                p0, p1 = jg // 2, (jg + g) // 2
                if p0 > 0 and p0 < NJP - NSCAL_PAIRS:
                    nc.sync.dma_start(xfT_sb[:, p0:p1], xfT[:, p0:p1])
                # x8p slabs ride the GpSimd ring so they never queue behind
                # the xfT stream on Sync
                nc.gpsimd.dma_start(x8p_sb[:, p0:p1], x8p[:, p0:p1])
                jg += g

            nc.scalar.dma_start(sr_sb[:], srt[:])
            nc.scalar.dma_start(wTh_sb[:], wTh[:])

            yps_t = None
            for ih in range(NIH):
                # ---- streaming j-loop: G^T -> exp -> d8 -> y^T/r accum ----
                # one tile per d-chunk: each accumulator must own a full PSUM
                # bank, since matmul start=True zeroes whole 2KB zero-regions
                yps_t = [yps.tile([DC, IHW], fp32, name=f"y_ps{ih}_{c}",
                                  tag=f"y{c}") for c in range(NDC)]
                r_ps = rps.tile([4, IHW], fp32, name=f"r_ps{ih}", tag="r")

                d8_tiles = {}

                def g_block(jc, ih=ih):
                    # fp8 DoubleRow: each matmul contracts 2 d-chunks (K=256)
                    jp, q = jc // 2, jc % 2
                    g = gps.tile([JC, IHW], fp32, name=f"g_ps{ih}_{jc}", tag="g")
                    for c2 in range(NDC // 2):
                        nc.tensor.matmul(
                            g[:],
                            xfT_sb[:, jp, 2 * c2:2 * c2 + 2, q * JC:(q + 1) * JC],
                            xbT_sb[:, ih, 2 * c2:2 * c2 + 2, :],
                            start=(c2 == 0), stop=(c2 == NDC // 2 - 1),
                            perf_mode=mybir.MatmulPerfMode.DoubleRow,
                        )
                    w = wpool.tile([JC, IHW], fp16, name=f"w_sb{ih}_{jc}", tag="w")
                    nc.scalar.activation(
                        w[:], g[:], mybir.ActivationFunctionType.Exp,
                        bias=sqb_sb[:, jc:jc + 1], scale=EXP_SCALE,
                    )
                    # mean-centered fp8 copy for the y-phase DoubleRow matmul
                    if q == 0:
                        d8_tiles[jp] = dpool.tile([JC, 2, IHW], fp8,
                                                  name=f"d8_{ih}_{jp}", tag="d8")
                    nc.vector.tensor_scalar_add(d8_tiles[jp][:, q, :], w[:], -1.0)
                    return w

                RACC = 16     # j-chunks of w pre-summed (on DVE) per r-matmul
                NQ = NJ // RACC

                def r_mm(q, wsum):
                    # softmax denominator: ones.T @ sum(w); the tree pre-sum
                    # runs on the otherwise-idle DVE, so PE pays one r-matmul
                    # per RACC j-chunks
                    nc.tensor.matmul(
                        r_ps[:], ones_sb[:], wsum[:],
                        start=(q == 0), stop=(q == NQ - 1),
                        skip_group_check=True,
                    )

                w_tiles = {0: g_block(0)}
                acc_tiles = {}   # (level, idx) -> partial sum tile
                pending_r = []

                def acc_put(level, idx, t, ih=ih):
                    # binary tree: level L holds sums of 2^L w tiles
                    if 2 ** level == RACC:
                        pending_r.append((idx, t))
                        return
                    if (level, idx ^ 1) in acc_tiles:
                        sib = acc_tiles.pop((level, idx ^ 1))
                        # top-level sums all stay live until the deferred
                        # r-matmuls run in the epilogue
                        bufs = NQ + 1 if 2 ** (level + 1) == RACC else 3
                        s = misc.tile([JC, IHW], fp16,
                                      name=f"acc{ih}_{level}_{idx}",
                                      tag=f"acc{level}", bufs=bufs)
                        nc.vector.tensor_add(s[:], sib[:], t[:])
                        acc_put(level + 1, idx // 2, s)
                    else:
                        acc_tiles[(level, idx)] = t

                for jc in range(NJ):
                    if ih == 0 and jc == 12:
                        # the last 1MB of xfT rides the otherwise-empty
                        # Scalar ring (one dispatch slot between two Exps);
                        # it lands by ~45us, before its ~60us deadline, and
                        # keeps the Sync ring from still streaming xfT at
                        # ~135us where it stalled the final pairs
                        nc.scalar.dma_start(xfT_sb[:, NJP - NSCAL_PAIRS:],
                                            xfT[:, NJP - NSCAL_PAIRS:])
                    # emit next G block first so PE never waits on ACT's exp
                    if jc + 1 < NJ:
                        w_tiles[jc + 1] = g_block(jc + 1)
                    w = w_tiles.pop(jc)
                    if jc % 2 == 1:
                        jp = jc // 2
                        for c in range(NDC):
                            nc.tensor.matmul(
                                yps_t[c][:],
                                x8p_sb[:, jp, :, c * DC:(c + 1) * DC],
                                d8_tiles[jp][:],
                                start=(jp == 0), stop=(jp == NJP - 1),
                                perf_mode=mybir.MatmulPerfMode.DoubleRow,
                                skip_group_check=True,
                            )
                        d8_tiles.pop(jp)
                    acc_put(0, jc, w)
                # all r-matmuls deferred here: they fill the PE idle while
                # ACT/DVE stage y^T out of PSUM
                while pending_r:
                    r_mm(*pending_r.pop(0))

                # ---- epilogue: stage y^T (+ colsum bias), r, head matmuls ----
                r_row = misc.tile([1, IHW], fp32, name=f"r_row{ih}", tag="r_row")
                nc.scalar.copy(r_row[:], r_ps[0:1, :])

                ysb = [ypool.tile([DC, IHW], fp16, name=f"ysb{ih}_{c}", tag=f"y{c}")
                       for c in range(NDC)]
                for c in range(NDC):
                    # split the staging copies across ACT and DVE; both add
                    # the exact colsum(x) chunk as a per-partition bias
                    if c < 2:
                        nc.scalar.activation(
                            ysb[c][:], yps_t[c][:],
                            mybir.ActivationFunctionType.Identity,
                            bias=sr_sb[:, c:c + 1])
                    else:
                        nc.vector.tensor_scalar_add(
                            ysb[c][:], yps_t[c][:], sr_sb[:, c:c + 1])

                def emit_rt():
                    # transpose r [1, IHW] -> [128, NT] via PE transpose-mode;
                    # reuse the r bank (free once r_row is copied out)
                    rt = rps.tile([128, IHW], fp32, name=f"rt{ih}", tag="r")
                    for t in range(NT):
                        nc.tensor.matmul(
                            rt[:, t:t + 1],
                            r_row[0:1, t * 128:(t + 1) * 128],
                            idone_sb[:],
                            is_transpose=True,
                            start=(t == 0), stop=(t == NT - 1),
                            skip_group_check=True,
                        )
                    recip = misc.tile([128, NT], fp32,
                                      name=f"recip{ih}", tag="recip")
                    nc.vector.reciprocal(recip[:], rt[:, 0:NT])
                    return recip

                def head_mms(t, hp):
                    for c in range(NDC):
                        nc.tensor.matmul(
                            hp[:],
                            ysb[c][:, t * 128:(t + 1) * 128],
                            wTh_sb[:, c, :],
                            start=(c == 0), stop=(c == NDC - 1),
                        )

                def head_out(t, hp, recip, ih=ih):
                    i0 = ih * IHW
                    osb = misc.tile([128, O], fp16, name=f"osb{ih}_{t}",
                                    tag="osb", bufs=4)
                    nc.vector.tensor_scalar_mul(osb[:], hp[:], recip[:, t:t + 1])
                    # alternate output queues so the final drain never
                    # serializes behind one sequencer; the very last tile
                    # splits across both queues to halve the exposed drain
                    r0 = i0 + t * 128
                    if ih == NIH - 1 and t == NT - 1:
                        nc.sync.dma_start(out[r0:r0 + 64, :], osb[0:64, :])
                        nc.gpsimd.dma_start(out[r0 + 64:r0 + 128, :], osb[64:128, :])
                    else:
                        q = nc.sync if t % 2 == 0 else nc.gpsimd
                        q.dma_start(out[r0:r0 + 128, :], osb[:],
                                    single_packet=True)

                recip = emit_rt()
                for t in range(NT):
                    # on the last half the g banks are free: double-buffer the
                    # head psum across hps/gps to overlap the i-tiles
                    if ih == NIH - 1 and t % 2 == 1:
                        hp = gps.tile([128, O], fp32, name=f"h_ps{ih}_{t}",
                                      tag="g")
                    else:
                        hp = hps.tile([128, O], fp32, name=f"h_ps{ih}_{t}",
                                      tag="h")
                    head_mms(t, hp)
                    head_out(t, hp, recip)

    nc.compile()
    return nc


def _get_nc():
    if "nc" not in _CACHE:
        _CACHE["nc"] = _build_nc()
    return _CACHE["nc"]


def kernel(x: np.ndarray, W: np.ndarray, b: np.ndarray) -> np.ndarray:
    from concourse import bass_utils

    x = np.asarray(x, dtype=np.float32)
    W = np.asarray(W, dtype=np.float32)
    b = np.asarray(b, dtype=np.float32)

    import concourse.mybir as mybir
    FP8 = mybir.dt.np(mybir.dt.float8e4)

    x8 = x.astype(FP8)
    # xfT [p, jp, c, jj] = x8[jp*256 + jj, c*128 + p]
    xfT_np = np.ascontiguousarray(
        x8.reshape(NJP, 2 * JC, NDC, DC).transpose(3, 0, 2, 1))
    # y-phase stationary: pairs layout [jp, j2, q, d] = x8[(2jp+q)*128 + j2, d]
    x8p_np = np.ascontiguousarray(
        x8.reshape(NJP, 2, JC, D).transpose(2, 0, 1, 3))
    wTh_np = np.ascontiguousarray(
        W.T.astype(np.float16).reshape(NDC, DC, O).transpose(1, 0, 2))
    # host-side softmax bias (from the quantized x, consistent with G) and
    # the exact fp32 column sum of the original x
    sq8 = (x8.astype(np.float32) ** 2).sum(axis=1)
    sqb_np = np.ascontiguousarray(
        ((ZSHIFT - sq8) * INV_2S2).astype(np.float32).reshape(NJ, JC).T)
    sr_np = np.ascontiguousarray(
        x.sum(axis=0, dtype=np.float64).astype(np.float32).reshape(NDC, DC).T)

    in_maps = []
    for k in range(NCORES):
        # xbT [p, ih, c, i] = x8[k*MB + ih*IHW + i, c*128 + p]
        xbT_np = np.ascontiguousarray(
            x8[k * MB:(k + 1) * MB].reshape(NIH, IHW, NDC, DC)
            .transpose(3, 0, 2, 1))
        in_maps.append({"xfT": xfT_np, "x8p": x8p_np, "xbT": xbT_np,
                        "wTh": wTh_np, "sqbt": sqb_np, "srt": sr_np})

    nc = _get_nc()
    br = bass_utils.run_bass_kernel_spmd(nc, in_maps, core_ids=list(range(NCORES)))
    _CACHE["last_results"] = br

    out = np.concatenate([br.results[k]["out"] for k in range(NCORES)], axis=0)
    return (out.astype(np.float32) + b[None, :])
